# revision 1
# baseline (speedup 1.0000x reference)
"""Trainium2 Bass kernel for windowed 32-pt FFT -> top-8 magnitude masking -> iFFT.

Per core (pure data parallel over batch), tiles of [128, 512] fp32:
  host pre-transposes x into freq-major tiles: partition 32g+n = freq n of
  row-group g, free col f = row 512g+f within the tile.
    -> PE matmul vs block-diag windowed DFT matrix (half spectrum packed:
       [Re_0..Re_16, Im_1..Im_15] per 32-partition group)
    -> ACT Square (PSUM -> SBUF)
    -> PE matmul vs 0/1 "pair add + reflect + bias" matrix:
       s32[k] = (1 - k*eta) * (Re_j^2 + Im_j^2), j = min(k, 32-k)
    -> ACT Sqrt (PSUM -> SBUF): biased magnitudes, freq-major
    -> DVE 32x32 block transpose to row-major
    -> DVE InstMax per [128,32] row-tile: sorted top-8 -> thresholds
    -> DVE is_ge vs broadcast 8th-largest, GPSIMD multiply: masked magnitudes
    -> DVE block transpose back to freq-major
    -> PE matmul vs cosine reconstruction matrix (1/32 and bias removal folded)
    -> DMA out (host inverse-permutes)

The multiplicative bias (1 - k*2^-20) makes otherwise bitwise-equal
conjugate-pair magnitudes strictly decreasing in k, so ">= 8th largest"
selects exactly 8 entries, ties broken toward lower k like jax.lax.top_k.
The cosine basis is symmetric under k -> 32-k, so pair-element choice
cannot change the output.
"""

import math

import numpy as np

B_TOTAL = 1048576
S = 32
N_CORES = 8
R_PER_CORE = B_TOTAL // N_CORES  # 131072
TILE_F = 512                     # rows per 32-partition group per tile
ROWS_PER_TILE = 4 * TILE_F       # 2048
N_TILES = R_PER_CORE // ROWS_PER_TILE  # 64
SEGS = TILE_F // 32              # 16
ETA = 2.0 ** -20

_cache = {}


def _build_consts():
    n = np.arange(S, dtype=np.float64)
    w = (0.5 - 0.5 * np.cos(2.0 * np.pi * np.arange(S, dtype=np.float32) / S))
    w = w.astype(np.float32).astype(np.float64)  # fp32 window values

    B32 = np.zeros((S, S), dtype=np.float64)
    for m in range(17):
        B32[:, m] = w * np.cos(2.0 * np.pi * m * n / S)
    for j in range(1, 16):
        B32[:, 16 + j] = -w * np.sin(2.0 * np.pi * j * n / S)

    c = 1.0 - np.arange(S, dtype=np.float64) * ETA

    Pm = np.zeros((S, S), dtype=np.float64)
    for kk in range(S):
        j = min(kk, S - kk)
        Pm[j, kk] = c[kk]
        if 1 <= j <= 15:
            Pm[16 + j, kk] = c[kk]

    Cm = np.zeros((S, S), dtype=np.float64)
    for kk in range(S):
        Cm[kk, :] = np.cos(2.0 * np.pi * kk * n / S) / (S * math.sqrt(c[kk]))

    def blockdiag4(M):
        out = np.zeros((128, 128), dtype=np.float32)
        for g in range(4):
            out[g * 32:(g + 1) * 32, g * 32:(g + 1) * 32] = M.astype(np.float32)
        return out

    return blockdiag4(B32), blockdiag4(Pm), blockdiag4(Cm)


def _build_program():
    import concourse.mybir as mybir
    from concourse import bacc
    from concourse.tile import TileContext

    f32 = mybir.dt.float32
    nc = bacc.Bacc("TRN2", target_bir_lowering=False, debug=False)

    x_d = nc.dram_tensor("x", [N_TILES, 128, TILE_F], f32, kind="ExternalInput")
    bm_d = nc.dram_tensor("Bm", [128, 128], f32, kind="ExternalInput")
    pm_d = nc.dram_tensor("Pm", [128, 128], f32, kind="ExternalInput")
    cm_d = nc.dram_tensor("Cm", [128, 128], f32, kind="ExternalInput")
    out_d = nc.dram_tensor("out", [N_TILES, 128, TILE_F], f32,
                           kind="ExternalOutput")

    x_v = x_d.ap()
    out_v = out_d.ap()

    with TileContext(nc) as tc:
        with (
            tc.tile_pool(name="consts", bufs=1) as cpool,
            tc.tile_pool(name="io", bufs=4) as io_pool,
            tc.tile_pool(name="work", bufs=4) as work_pool,
            tc.tile_pool(name="psum", bufs=2, space="PSUM") as psum_pool,
        ):
            bm = cpool.tile([128, 128], f32, tag="bm")
            pm = cpool.tile([128, 128], f32, tag="pm")
            cm = cpool.tile([128, 128], f32, tag="cm")
            nc.sync.dma_start(bm[:], bm_d.ap())
            nc.sync.dma_start(pm[:], pm_d.ap())
            nc.sync.dma_start(cm[:], cm_d.ap())

            # Pairs of tiles share double-width row-major buffers so the
            # DVE transposes and mask passes run at [128, 1024] (half the
            # instruction count / per-op SBUF bubbles). Matmuls, ACT, and
            # PSUM stay per-[128, 512].
            W = 2 * TILE_F
            SEG2 = 2 * SEGS
            for j in range(N_TILES // 2):
                mag_rm = work_pool.tile([128, W], f32, tag="mag_rm")
                for h in (0, 1):
                    i = 2 * j + h
                    x_t = io_pool.tile([128, TILE_F], f32, tag="x_t")
                    nc.sync.dma_start(x_t[:], x_v[i])

                    g_ps = psum_pool.tile([128, TILE_F], f32, tag="g")
                    nc.tensor.matmul(g_ps[:], bm[:], x_t[:],
                                     start=True, stop=True)

                    sq = work_pool.tile([128, TILE_F], f32, tag="sq")
                    nc.scalar.square(sq[:], g_ps[:])

                    s_ps = psum_pool.tile([128, TILE_F], f32, tag="s")
                    nc.tensor.matmul(s_ps[:], pm[:], sq[:],
                                     start=True, stop=True)

                    mag_t = work_pool.tile([128, TILE_F], f32, tag="mag_t")
                    nc.scalar.sqrt(mag_t[:], s_ps[:])

                    nc.vector.transpose(
                        mag_rm[:, TILE_F * h:TILE_F * (h + 1)], mag_t[:]
                    )

                th8 = work_pool.tile([128, 8 * SEG2], f32, tag="th8")
                for t in range(SEG2):
                    nc.vector.max(
                        out=th8[:, 8 * t:8 * t + 8],
                        in_=mag_rm[:, 32 * t:32 * t + 32],
                    )

                th_b = th8[:, 7:8 * SEG2:8].to_broadcast([128, SEG2, 32])
                mag3 = mag_rm[:].rearrange("p (t n) -> p t n", n=32)

                mask = work_pool.tile([128, W], f32, tag="mask")
                mask3 = mask[:].rearrange("p (t n) -> p t n", n=32)
                nc.vector.tensor_tensor(
                    mask3, mag3, th_b, op=mybir.AluOpType.is_ge
                )

                coef_rm = work_pool.tile([128, W], f32, tag="coef_rm")
                nc.vector.tensor_mul(coef_rm[:], mask[:], mag_rm[:])

                coef_t = work_pool.tile([128, W], f32, tag="coef_t")
                nc.vector.transpose(coef_t[:], coef_rm[:])

                for h in (0, 1):
                    i = 2 * j + h
                    o_ps = psum_pool.tile([128, TILE_F], f32, tag="o")
                    nc.tensor.matmul(
                        o_ps[:], cm[:],
                        coef_t[:, TILE_F * h:TILE_F * (h + 1)],
                        start=True, stop=True,
                    )

                    o_sb = io_pool.tile([128, TILE_F], f32, tag="o_sb")
                    nc.scalar.copy(o_sb[:], o_ps[:])

                    nc.sync.dma_start(out_v[i], o_sb[:])

    nc.compile()
    return nc


def _get_program():
    if "nc" not in _cache:
        _cache["nc"] = _build_program()
        _cache["consts"] = _build_consts()
    return _cache["nc"], _cache["consts"]


def _pre_permute(xc: np.ndarray) -> np.ndarray:
    # [R_PER_CORE, 32] -> [N_TILES, 128, TILE_F]
    # tile i, partition 32g+n, col f  <->  row 2048 i + 512 g + f, freq n
    t = xc.reshape(N_TILES, 4, TILE_F, S)          # [i, g, f, n]
    return np.ascontiguousarray(t.transpose(0, 1, 3, 2)).reshape(
        N_TILES, 128, TILE_F
    )


def _post_permute(op: np.ndarray) -> np.ndarray:
    # [N_TILES, 128, TILE_F] -> [R_PER_CORE, 32]
    t = op.reshape(N_TILES, 4, S, TILE_F).transpose(0, 1, 3, 2)  # [i, g, f, n]
    return np.ascontiguousarray(t).reshape(R_PER_CORE, S)


def kernel(x: np.ndarray) -> np.ndarray:
    from concourse.bass_utils import run_bass_kernel_spmd

    nc, (bm, pm, cm) = _get_program()

    xc = np.ascontiguousarray(x[:, :, 0], dtype=np.float32)  # [B, 32]
    shards = xc.reshape(N_CORES, R_PER_CORE, S)
    in_maps = [
        {"x": _pre_permute(shards[c]), "Bm": bm, "Pm": pm, "Cm": cm}
        for c in range(N_CORES)
    ]
    res = run_bass_kernel_spmd(nc, in_maps, core_ids=list(range(N_CORES)))
    out = np.concatenate(
        [_post_permute(r["out"]) for r in res.results], axis=0
    )
    return out.reshape(B_TOTAL, S, 1).astype(np.float32)



# revision 3
# speedup vs baseline: 4.0533x; 4.0533x over previous
"""Trainium2 Bass kernel v3: windowed 32-pt FFT -> top-8 mask -> iFFT.

Engine-balanced rework of the baseline:
  PE  : Bm matmul (fp32, exact selection basis), Pm pair-add (f32r),
        mag transpose to row-major (fp32, 4x 128-blocks), coef transpose
        back (bf16), Cm reconstruction (f32r x bf16).
  ACT : square (f32r out), sqrt (fp32), final PSUM->SBUF copy.
  Pool: magT PSUM->SBUF copy, is_ge vs 8th-largest, mask*mag (bf16 out),
        coefT PSUM->SBUF copy.
  DVE : 16x InstMax per tile only.

Bias (1 - k*eta) folded into Bm/Pm as in the baseline; Cm corrects.
"""

import math

import numpy as np

B_TOTAL = 1048576
S = 32
N_CORES = 8
R_PER_CORE = B_TOTAL // N_CORES  # 131072
TILE_F = 512
ROWS_PER_TILE = 4 * TILE_F       # 2048
N_TILES = R_PER_CORE // ROWS_PER_TILE  # 64
SEGS = TILE_F // 32              # 16
ETA = 2.0 ** -20

_cache = {}


def _round_f32r(a):
    """RNE to 11 mantissa bits — matches TRN2 FP32r operand rounding."""
    u = np.asarray(a, np.float32).view(np.uint32).astype(np.uint64)
    drop = 12
    lsb = (u >> drop) & 1
    r = (u + (1 << (drop - 1)) - 1 + lsb) & ~np.uint64((1 << drop) - 1)
    return r.astype(np.uint32).view(np.float32)


def _build_consts():
    n = np.arange(S, dtype=np.float64)
    w = (0.5 - 0.5 * np.cos(2.0 * np.pi * np.arange(S, dtype=np.float32) / S))
    w = w.astype(np.float32).astype(np.float64)

    B32 = np.zeros((S, S), dtype=np.float64)
    for m in range(17):
        B32[:, m] = w * np.cos(2.0 * np.pi * m * n / S)
    for j in range(1, 16):
        B32[:, 16 + j] = -w * np.sin(2.0 * np.pi * j * n / S)

    c = 1.0 - np.arange(S, dtype=np.float64) * ETA

    # pure 0/1 pair-add matrix — exactly representable in f32r; the
    # tie-breaking bias c is applied as the ACT sqrt's per-partition scale
    Pm = np.zeros((S, S), dtype=np.float64)
    for kk in range(S):
        j = min(kk, S - kk)
        Pm[j, kk] = 1.0
        if 1 <= j <= 15:
            Pm[16 + j, kk] = 1.0

    Cm = np.zeros((S, S), dtype=np.float64)
    for kk in range(S):
        Cm[kk, :] = np.cos(2.0 * np.pi * kk * n / S) / (S * math.sqrt(c[kk]))

    def blockdiag4(M, dtype=np.float32):
        out = np.zeros((128, 128), dtype=np.float32)
        for g in range(4):
            out[g * 32:(g + 1) * 32, g * 32:(g + 1) * 32] = M.astype(np.float32)
        return out

    bm = blockdiag4(B32)
    pm = _round_f32r(blockdiag4(Pm))
    import ml_dtypes
    cm = blockdiag4(Cm).astype(ml_dtypes.bfloat16)
    idf = np.eye(128, dtype=np.float32)
    idb = np.eye(128, dtype=ml_dtypes.bfloat16)
    cvec = np.tile(c.astype(np.float32), 4).reshape(128, 1)
    return bm, pm, cm, idf, idb, cvec


def _build_program():
    import concourse.mybir as mybir
    from concourse import bacc
    from concourse.tile import TileContext

    f32 = mybir.dt.float32
    f32r = mybir.dt.float32r
    bf16 = mybir.dt.bfloat16
    nc = bacc.Bacc("TRN2", target_bir_lowering=False, debug=False)

    x_d = nc.dram_tensor("x", [N_TILES, 128, TILE_F], f32, kind="ExternalInput")
    bm_d = nc.dram_tensor("Bm", [128, 128], f32, kind="ExternalInput")
    pm_d = nc.dram_tensor("Pm", [128, 128], f32r, kind="ExternalInput")
    cm_d = nc.dram_tensor("Cm", [128, 128], bf16, kind="ExternalInput")
    idf_d = nc.dram_tensor("Idf", [128, 128], f32, kind="ExternalInput")
    idb_d = nc.dram_tensor("Idb", [128, 128], bf16, kind="ExternalInput")
    cv_d = nc.dram_tensor("Cv", [128, 1], f32, kind="ExternalInput")
    out_d = nc.dram_tensor("out", [N_TILES, 128, TILE_F], f32,
                           kind="ExternalOutput")

    x_v = x_d.ap()
    out_v = out_d.ap()

    with TileContext(nc) as tc:
        with (
            tc.tile_pool(name="consts", bufs=1) as cpool,
            tc.tile_pool(name="io", bufs=4) as io_pool,
            tc.tile_pool(name="work", bufs=3) as work_pool,
            tc.tile_pool(name="psA", bufs=2, space="PSUM") as psA,
            tc.tile_pool(name="psB", bufs=2, space="PSUM") as psB,
            tc.tile_pool(name="psT", bufs=1, space="PSUM") as psT,
            tc.tile_pool(name="psC", bufs=1, space="PSUM") as psC,
            tc.tile_pool(name="psO", bufs=2, space="PSUM") as psO,
        ):
            bm = cpool.tile([128, 128], f32, tag="bm")
            pm = cpool.tile([128, 128], f32r, tag="pm")
            cm = cpool.tile([128, 128], bf16, tag="cm")
            idf = cpool.tile([128, 128], f32, tag="idf")
            idb = cpool.tile([128, 128], bf16, tag="idb")
            cv = cpool.tile([128, 1], f32, tag="cv")
            nc.sync.dma_start(bm[:], bm_d.ap())
            nc.sync.dma_start(pm[:], pm_d.ap())
            nc.sync.dma_start(cm[:], cm_d.ap())
            nc.sync.dma_start(idf[:], idf_d.ap())
            nc.sync.dma_start(idb[:], idb_d.ap())
            nc.sync.dma_start(cv[:], cv_d.ap())

            W = 2 * TILE_F
            SEG2 = 2 * SEGS
            for j in range(N_TILES // 2):
                mag_rm = work_pool.tile([128, W], f32, tag="mag_rm")
                for h in (0, 1):
                    i = 2 * j + h
                    x_t = io_pool.tile([128, TILE_F], f32, tag="x_t")
                    nc.sync.dma_start(x_t[:], x_v[i])

                    g_ps = psA.tile([128, TILE_F], f32, tag="g")
                    nc.tensor.matmul(g_ps[:], bm[:], x_t[:],
                                     start=True, stop=True)

                    sq = work_pool.tile([128, TILE_F], f32r, tag="sq")
                    nc.scalar.square(sq[:], g_ps[:])

                    s_ps = psB.tile([128, TILE_F], f32, tag="s")
                    nc.tensor.matmul(s_ps[:], pm[:], sq[:],
                                     start=True, stop=True)

                    mag_t = work_pool.tile([128, TILE_F], f32, tag="mag_t")
                    nc.scalar.activation(
                        mag_t[:], s_ps[:],
                        mybir.ActivationFunctionType.Sqrt,
                        scale=cv[:],
                    )

                    # PE transpose to row-major: 4 x [128,128] blocks
                    magT_ps = psT.tile([128, TILE_F], f32, tag="magT")
                    for b in range(4):
                        nc.tensor.transpose(
                            magT_ps[:, 128 * b:128 * (b + 1)],
                            mag_t[:, 128 * b:128 * (b + 1)],
                            idf[:],
                        )
                    nc.gpsimd.tensor_copy(
                        mag_rm[:, TILE_F * h:TILE_F * (h + 1)], magT_ps[:]
                    )

                th8 = work_pool.tile([128, 8 * SEG2], f32, tag="th8")
                for t in range(SEG2):
                    nc.vector.max(
                        out=th8[:, 8 * t:8 * t + 8],
                        in_=mag_rm[:, 32 * t:32 * t + 32],
                    )

                th_b = th8[:, 7:8 * SEG2:8].to_broadcast([128, SEG2, 32])
                mag3 = mag_rm[:].rearrange("p (t n) -> p t n", n=32)

                mask = work_pool.tile([128, W], f32, tag="mask")
                mask3 = mask[:].rearrange("p (t n) -> p t n", n=32)
                nc.gpsimd.tensor_tensor(
                    mask3, mag3, th_b, op=mybir.AluOpType.is_ge
                )

                coef_rm = work_pool.tile([128, W], bf16, tag="coef_rm")
                nc.gpsimd.tensor_mul(coef_rm[:], mask[:], mag_rm[:])

                # PE transpose coef back to freq-major (bf16), 8 blocks
                coefT_ps = psC.tile([128, W], bf16, tag="coefT")
                for b in range(8):
                    nc.tensor.transpose(
                        coefT_ps[:, 128 * b:128 * (b + 1)],
                        coef_rm[:, 128 * b:128 * (b + 1)],
                        idb[:],
                    )
                coef_t = work_pool.tile([128, W], bf16, tag="coef_t")
                nc.gpsimd.tensor_copy(coef_t[:], coefT_ps[:])

                for h in (0, 1):
                    i = 2 * j + h
                    o_ps = psO.tile([128, TILE_F], f32, tag="o")
                    nc.tensor.matmul(
                        o_ps[:], cm[:],
                        coef_t[:, TILE_F * h:TILE_F * (h + 1)],
                        start=True, stop=True,
                    )

                    o_sb = io_pool.tile([128, TILE_F], f32, tag="o_sb")
                    nc.scalar.copy(o_sb[:], o_ps[:])

                    nc.sync.dma_start(out_v[i], o_sb[:])

    nc.compile()
    return nc


def _get_program():
    if "nc" not in _cache:
        _cache["nc"] = _build_program()
        _cache["consts"] = _build_consts()
    return _cache["nc"], _cache["consts"]


def _pre_permute(xc: np.ndarray) -> np.ndarray:
    t = xc.reshape(N_TILES, 4, TILE_F, S)          # [i, g, f, n]
    return np.ascontiguousarray(t.transpose(0, 1, 3, 2)).reshape(
        N_TILES, 128, TILE_F
    )


def _post_permute(op: np.ndarray) -> np.ndarray:
    t = op.reshape(N_TILES, 4, S, TILE_F).transpose(0, 1, 3, 2)
    return np.ascontiguousarray(t).reshape(R_PER_CORE, S)


def kernel(x: np.ndarray) -> np.ndarray:
    from concourse.bass_utils import run_bass_kernel_spmd

    nc, (bm, pm, cm, idf, idb, cvec) = _get_program()

    xc = np.ascontiguousarray(x[:, :, 0], dtype=np.float32)  # [B, 32]
    shards = xc.reshape(N_CORES, R_PER_CORE, S)
    in_maps = [
        {"x": _pre_permute(shards[c]), "Bm": bm, "Pm": pm, "Cm": cm,
         "Idf": idf, "Idb": idb, "Cv": cvec}
        for c in range(N_CORES)
    ]
    res = run_bass_kernel_spmd(nc, in_maps, core_ids=list(range(N_CORES)))
    out = np.concatenate(
        [_post_permute(r["out"]) for r in res.results], axis=0
    )
    return out.reshape(B_TOTAL, S, 1).astype(np.float32)
